# revision 19
# baseline (speedup 1.0000x reference)
"""MAB (Set-Transformer multihead attention block) Trainium2 Bass kernel, v4.

Reference math (fp32):
  Q = q @ Wq.T + bq ; K = k @ Wk.T + bk ; V = k @ Wv.T + bv    [B,N,256]
  per head h (8 heads x 32): s = Qh @ Kh.T / 16 ; a = softmax(s)
  Oh = Qh + a @ Vh ; o = concat(Oh) ; o = LN0(o) ; o = o + relu(o @ Wo.T + bo)
  out = LN1(o)

Sharding: 8 cores = (batch b in 0..3, query-half in 0..1); no collectives.

v4 vs v3:
  - q/k/W are pre-transposed and pre-cast to bf16 on the host (input
    layout prep in the kernel() wrapper): no on-device input transposes
    or casts, and half the DMA bytes. Device inputs are qT/kT/W*T bf16.
  - epilogue rewritten: every matmul operand bf16 (no more fp32
    LOW_HIGH two-pass matmuls anywhere), LN0 broadcast tensors drained
    to SBUF bf16 so the apply runs at DVE 2x, and the two LN/FFN chunks
    are emitted stage-interleaved so their serial chains overlap.
  - attention unchanged from v3: 4-way row-tiled concurrent score
    matmuls, PV/denominator col-tiled matmuls interleaved at lag 1,
    exp split ACT (table exp) / DVE (bf16 bit-trick), denominators via
    ones matmuls, 1/d via reciprocal_approx_fast, rstd via rsqrt
    bit-trick + Newton (single ACT table load in the whole kernel).
"""

import os
import sys
from contextlib import ExitStack

import numpy as np

for _p in ("/opt/trn_rl_repo", "/root/.axon_site/_ro/trn_rl_repo"):
    if os.path.isdir(_p) and _p not in sys.path:
        sys.path.insert(0, _p)

import ml_dtypes  # noqa: E402
import concourse.bass as bass  # noqa: E402
import concourse.tile as tile  # noqa: E402
from concourse import bacc, mybir  # noqa: E402
from concourse.masks import make_identity  # noqa: E402

F32 = mybir.dt.float32
BF16 = mybir.dt.bfloat16
I16 = mybir.dt.int16
I32 = mybir.dt.int32
P = 128
EPS = 1e-5

AF = mybir.ActivationFunctionType
OP = mybir.AluOpType

LOG2E = 1.4426950408889634
# exp(x/16) ~= bf16(bits = round(x*EXP_A + EXP_B))
EXP_A = 128.0 * LOG2E / 16.0
EXP_B = 128.0 * (127.0 - 0.04380)
# rsqrt(v) ~= bf16(bits = round(bits32(v)*RS_A + RS_B)), then 1 Newton step
RS_A = -64.0 / (1 << 23)
RS_B = 24375.283445

ACT_EXP_SHARE = 0.61  # fraction of exp drains on ScalarE


class Cfg:
    def __init__(self, NQ=1024, NK=2048, D=256, H=8, ln0_gb=False, ln1_gb=False):
        self.NQ, self.NK, self.D, self.H = NQ, NK, D, H
        self.HD = D // H            # 32
        self.DO = D // P            # 2
        self.QT = NQ // P           # 8
        self.KT = NK // P           # 16
        self.SC = 512               # score/attn q-chunk
        self.QCN = NQ // self.SC    # 2
        self.LC = 512               # ln/ffn q-chunk
        self.LCN = NQ // self.LC
        self.ln0_gb = ln0_gb        # apply non-trivial g0/b0
        self.ln1_gb = ln1_gb
        assert self.HD == 32 and self.DO == 2


class EngineSplit:
    """Bresenham-style assigner: returns True for ACT with given share."""

    def __init__(self, share):
        self.share = share
        self.acc = 0.0

    def take(self):
        self.acc += self.share
        if self.acc >= 1.0:
            self.acc -= 1.0
            return True
        return False


def _emit(nc: bass.Bass, tc: tile.TileContext, ctx: ExitStack, io: dict, cfg: Cfg):
    NQ, NK, D, H = cfg.NQ, cfg.NK, cfg.D, cfg.H
    DO, QT, KT, SC, LC = cfg.DO, cfg.QT, cfg.KT, cfg.SC, cfg.LC

    const = ctx.enter_context(tc.tile_pool(name="const", bufs=1))
    persist = ctx.enter_context(tc.tile_pool(name="persist", bufs=1))

    # ---- constants ----
    ident = const.tile([P, P], F32)
    make_identity(nc, ident)
    ident_bf = const.tile([P, P], BF16)
    nc.vector.tensor_copy(ident_bf, ident)
    ones32 = const.tile([P, 32], BF16)
    nc.vector.memset(ones32, 1.0)
    ones_r = const.tile([1, 512], BF16)  # ones row: bias-matmul rhs / lhsT
    nc.vector.memset(ones_r, 1.0)
    ones_k = const.tile([P, 1], BF16)    # LN stats lhsT (partition sum)
    nc.vector.memset(ones_k, 1.0)

    def vec_row(name, dtype=BF16):
        tf = const.tile([1, D], F32, name=f"{name}_rowf")
        nc.sync.dma_start(tf, io[name][:].rearrange("(o d) -> o d", o=1))
        if dtype == F32:
            return tf
        t = const.tile([1, D], BF16, name=f"{name}_row")
        nc.vector.tensor_copy(t, tf)
        return t

    def vec_pm(name):
        t = const.tile([P, DO], F32, name=f"{name}_pm")
        nc.sync.dma_start(t, io[name][:].rearrange("(o p) -> p o", p=P))
        return t

    bq_pm, bk_pm, bo_pm = vec_pm("bq"), vec_pm("bk"), vec_pm("bo")
    bv_bc = const.tile([P, D], F32, name="bv_bc")
    nc.gpsimd.partition_broadcast(bv_bc, vec_row("bv", F32))
    g0_pm = vec_pm("g0") if cfg.ln0_gb else None
    b0_pm = vec_pm("b0") if cfg.ln0_gb else None
    if cfg.ln1_gb:
        g1_bc = const.tile([P, D], F32)
        nc.gpsimd.partition_broadcast(g1_bc, vec_row("g1", F32))
        b1_bc = const.tile([P, D], F32)
        nc.gpsimd.partition_broadcast(b1_bc, vec_row("b1", F32))

    # ---- persistent tensors (all bf16) ----
    Q_bf = persist.tile([P, DO, NQ], BF16, name="Q_bf")
    K_bf = persist.tile([P, DO, NK], BF16, name="K_bf")
    V_nat = persist.tile([P, KT, H, 32], BF16, name="V_nat")  # [tok, dv]/tile
    woT = persist.tile([P, DO, D], BF16, name="woT")
    O_bf = persist.tile([P, DO, NQ], BF16, name="O_bf")
    X0 = persist.tile([P, DO, NQ], BF16, name="X0")
    X1 = persist.tile([P, DO, NQ], BF16, name="X1")
    out_nat = persist.tile([P, QT, D], F32, name="out_nat")

    drain_split = EngineSplit(0.5)

    def drain(dst, src):
        if drain_split.take():
            nc.scalar.copy(dst, src)
        else:
            nc.vector.tensor_copy(dst, src)

    # ============ prologue: load pre-transposed inputs ============
    ph0 = ctx.enter_context(tc.tile_pool(name="ph0", bufs=1))

    # kT first (longest dependency chain: kT -> K/V proj -> attention)
    k_T = ph0.tile([P, DO, NK], BF16, name="k_T")
    for c0 in range(0, NK, NK // 2):
        nc.sync.dma_start(
            k_T[:, :, c0:c0 + NK // 2],
            io["kT"][:, c0:c0 + NK // 2].rearrange("(o p) t -> p o t", p=P))
    wT = {}
    for n in ("WkT", "WvT", "WqT", "WoT"):
        t = ph0.tile([P, DO, D], BF16, name=n) if n != "WoT" else woT
        nc.sync.dma_start(t, io[n][:].rearrange("(o p) f -> p o f", p=P))
        wT[n] = t
    q_T = ph0.tile([P, DO, NQ], BF16, name="q_T")
    nc.sync.dma_start(q_T, io["qT"][:].rearrange("(o p) t -> p o t", p=P))

    def proj_group(w, src, b_pm, dst, o, c0, ps):
        for ki in range(DO):
            nc.tensor.matmul(
                ps, lhsT=w[:, ki, o * P:(o + 1) * P],
                rhs=src[:, ki, c0:c0 + 512],
                start=(ki == 0), stop=(ki == DO - 1))
        nc.vector.tensor_scalar(dst[:, o, c0:c0 + 512], ps, 1.0,
                                b_pm[:, o:o + 1], OP.mult, OP.add)

    # o=0 projections of K and Q gate the first score matmuls: do them
    # in a short serial pre-phase; everything else rides inside block 0.
    with tc.tile_pool(name="pps", bufs=1, space="PSUM") as pps:
        for c in range(4):
            ps = pps.tile([P, 512], F32, tag=f"pj{c % 2}", name="pj")
            proj_group(wT["WkT"], k_T, bk_pm, K_bf, 0, c * 512, ps)
        for c in range(2):
            ps = pps.tile([P, 512], F32, tag=f"pj{c % 2}", name="pj")
            proj_group(wT["WqT"], q_T, bq_pm, Q_bf, 0, c * 512, ps)

    # =================== attention ===================
    exp_split = EngineSplit(ACT_EXP_SHARE)
    with tc.tile_pool(name="attn_sb", bufs=1) as asb, \
         tc.tile_pool(name="sps", bufs=1, space="PSUM") as sps, \
         tc.tile_pool(name="ops", bufs=1, space="PSUM") as ops, \
         tc.tile_pool(name="nrm", bufs=2) as nrm:
        attn01 = asb.tile([P, KT, 2, SC], BF16, name="attn01")
        attn23 = asb.tile([P, KT, 2, SC], BF16, name="attn23")

        # leftover projections, emitted one group per kt-step of block 0
        # (they use the sp2 score buffers, which block 0 leaves free)
        extras = []
        for t in range(KT):
            def vproj(t=t):
                ps = sps.tile([P, 2, SC], F32, tag="sp2", name="pvv")
                psv = ps[:, 0, 0:D]
                for ki in range(DO):
                    nc.tensor.matmul(
                        psv, lhsT=k_T[:, ki, t * P:(t + 1) * P],
                        rhs=wT["WvT"][:, ki, :],
                        start=(ki == 0), stop=(ki == DO - 1))
                nc.vector.tensor_tensor(
                    V_nat[:, t, :, :].rearrange("p h w -> p (h w)"), psv,
                    bv_bc, OP.add)
            extras.append(vproj)
        for c in range(4):
            def kproj(c=c):
                ps = sps.tile([P, 2, SC], F32, tag="sp2", name="pvk")
                proj_group(wT["WkT"], k_T, bk_pm, K_bf, 1, c * 512,
                           ps[:, 1, :])
            extras.append(kproj)
        for c in range(2):
            def qproj(c=c):
                ps = sps.tile([P, 2, SC], F32, tag="sp2", name="pvq")
                proj_group(wT["WqT"], q_T, bq_pm, Q_bf, 1, c * 512,
                           ps[:, 1, :])
            extras.append(qproj)

        # flat (block, kt) pipeline: PV matmuls lag LAG steps behind the
        # score/exp stream, so the exp engines are never starved by a PV
        # tail and the PE fills exp-pacing gaps.
        LAG = 4
        pvq = []
        blocks = [(qc, o) for qc in range(cfg.QCN) for o in range(DO)]
        for bi, (qc, o) in enumerate(blocks):
            qsl = slice(qc * SC, (qc + 1) * SC)
            blk0 = bi == 0
            state = {}

            def pv_step(t, o=o, state=state):
                if "po" not in state:
                    state["po"] = ops.tile([P, SC], F32, tag="po", name="po")
                    state["pd"] = ops.tile([P, SC], F32, tag="pd", name="pd")
                po, pd = state["po"], state["pd"]
                for m in range(4):
                    at = attn01 if m < 2 else attn23
                    rhs = at[:, t, m % 2, :]
                    nc.tensor.matmul(
                        po[32 * m:32 * m + 32, :],
                        lhsT=V_nat[:, t, o * 4 + m, :], rhs=rhs,
                        start=(t == 0), stop=(t == KT - 1),
                        tile_position=(0, 32 * m), skip_group_check=True)
                for m in range(4):
                    at = attn01 if m < 2 else attn23
                    rhs = at[:, t, m % 2, :]
                    nc.tensor.matmul(
                        pd[32 * m:32 * m + 32, :], lhsT=ones32, rhs=rhs,
                        start=(t == 0), stop=(t == KT - 1),
                        tile_position=(0, 32 * m), skip_group_check=True)

            def norm(qsl=qsl, o=o, state=state):
                rec = nrm.tile([P, SC], F32, tag="rec", name="rec")
                nc.vector.reciprocal_approx_fast(rec, state["pd"])
                osl = O_bf[:, o, qsl]
                nc.vector.tensor_tensor(osl, state["po"], rec, OP.mult)
                nc.vector.tensor_tensor(osl, osl, Q_bf[:, o, qsl], OP.add)

            for kt in range(KT):
                if blk0:
                    extras.pop(0)()          # V tile kt
                    if extras and kt >= KT - 7:
                        extras.pop(0)()      # K/Q o=1 projections
                ksl = slice(kt * P, (kt + 1) * P)
                pj = (2 * kt) % (2 if blk0 else 3)
                pa = sps.tile([P, 2, SC], F32, tag=f"sp{pj}", name="pa")
                pb = sps.tile([P, 2, SC], F32,
                              tag=f"sp{(pj + 1) % (2 if blk0 else 3)}",
                              name="pb")
                for m, (pt, sl) in enumerate(
                        ((pa, 0), (pa, 1), (pb, 0), (pb, 1))):
                    nc.tensor.matmul(
                        pt[:, sl, :],
                        lhsT=K_bf[32 * m:32 * m + 32, o, ksl],
                        rhs=Q_bf[32 * m:32 * m + 32, o, qsl],
                        start=True, stop=True,
                        tile_position=(32 * m, 0))
                for at, pt in ((attn01, pa), (attn23, pb)):
                    if exp_split.take():
                        nc.scalar.activation(at[:, kt, :, :], pt, AF.Exp,
                                             scale=1.0 / 16.0)
                    else:
                        nc.vector.tensor_scalar(
                            at[:, kt, :, :].bitcast(I16), pt,
                            EXP_A, EXP_B, OP.mult, OP.add)
                if kt == KT - 1:
                    pvq.append(lambda t=kt, f=pv_step, n=norm: (f(t), n()))
                else:
                    pvq.append(lambda t=kt, f=pv_step: f(t))
                while len(pvq) > LAG:
                    pvq.pop(0)()
        while pvq:
            pvq.pop(0)()

    # =================== epilogue: LN0, FFN, LN1, out ===================
    # stage-interleaved across the two LC chunks so serial chains overlap
    with tc.tile_pool(name="ep_sb", bufs=1) as esb, \
         tc.tile_pool(name="st_ps", bufs=1, space="PSUM") as stp, \
         tc.tile_pool(name="bc_ps", bufs=1, space="PSUM") as bcp, \
         tc.tile_pool(name="f_ps", bufs=1, space="PSUM") as fps, \
         tc.tile_pool(name="o_ps", bufs=1, space="PSUM") as otp:
        NC = LC // P  # 4 query-pieces per chunk
        csls = [slice(c * LC, (c + 1) * LC) for c in range(cfg.LCN)]

        def ln_stats(src, csl, tag):
            """Stage 1: x^2 + ones-matmul stats -> st_sb [1, 2, LC] bf16."""
            x2 = esb.tile([P, DO, LC], BF16, tag=f"x2{tag}", name="x2")
            for o in range(DO):
                nc.scalar.activation(x2[:, o, :], src[:, o, csl], AF.Square)
            st_sb = esb.tile([1, 2, LC], BF16, tag=f"stsb{tag}", name="st_sb")
            for i in range(2):
                st = stp.tile([1, LC], F32, tag="st", name="st")
                for o in range(DO):
                    rhs = src[:, o, csl] if i == 0 else x2[:, o, :]
                    nc.tensor.matmul(st, lhsT=ones_k, rhs=rhs,
                                     start=(o == 0), stop=(o == DO - 1))
                nc.scalar.copy(st_sb[:, i, :], st)
            return st_sb

        def ln_rows(st_sb, tag, bf_out=False):
            """Stage 2: transpose stats pieces, row math -> A8/B8 [128, NC]."""
            st_t = stp.tile([P, 2 * NC], F32, tag="stt", name="st_t")
            for i in range(2):
                for j in range(NC):
                    nc.tensor.matmul(
                        st_t[:, i * NC + j:i * NC + j + 1],
                        lhsT=st_sb[0:1, i, j * P:(j + 1) * P],
                        rhs=ones_r[0:1, 0:1], start=True, stop=True)
            stt_sb = esb.tile([P, 2 * NC], F32, tag=f"sttsb{tag}", name="stt_sb")
            nc.vector.tensor_copy(stt_sb, st_t)
            sx, sx2 = stt_sb[:, 0:NC], stt_sb[:, NC:2 * NC]
            r8 = esb.tile([P, 5, NC], F32, tag=f"r8{tag}", name="r8")
            mu, ve, var_e, y2t, A8 = (r8[:, i, :] for i in range(5))
            nc.vector.tensor_scalar(mu, sx, 1.0 / D, None, OP.mult)
            nc.vector.tensor_scalar(ve, sx2, 1.0 / D, EPS, OP.mult, OP.add)
            nc.vector.tensor_tensor(var_e, mu, mu, OP.mult)
            nc.vector.tensor_tensor(var_e, ve, var_e, OP.subtract)
            y0 = esb.tile([P, NC], I16, tag=f"y0{tag}", name="y0")
            nc.vector.tensor_scalar(y0, var_e[:].bitcast(I32),
                                    RS_A, RS_B, OP.mult, OP.add)
            y0b = y0[:].bitcast(BF16)  # ~= rstd seed (+-3.7%)
            nc.vector.tensor_tensor(y2t, y0b, y0b, OP.mult)
            nc.vector.tensor_tensor(y2t, y2t, var_e, OP.mult)
            nc.vector.tensor_scalar(y2t, y2t, -0.5, 1.5, OP.mult, OP.add)
            nc.vector.tensor_tensor(A8, y2t, y0b, OP.mult)  # rstd
            B8 = r8[:, 3, :]  # reuse y2t slot
            nc.vector.scalar_tensor_tensor(B8, mu, -1.0, A8, OP.mult, OP.mult)
            if not bf_out:
                return A8, B8
            ab = esb.tile([P, 2, NC], BF16, tag=f"ab{tag}", name="ab")
            nc.vector.tensor_copy(ab[:, 0, :], A8)
            nc.vector.tensor_copy(ab[:, 1, :], B8)
            return ab

        # ---- LN0 ----
        st0 = [ln_stats(O_bf, csls[c], f"a{c}") for c in range(cfg.LCN)]
        ab0 = [ln_rows(st0[c], f"a{c}", bf_out=True) for c in range(cfg.LCN)]
        pab_sb = []
        for c in range(cfg.LCN):
            # transpose A/B pieces to partition-0 rows, then gpsimd
            # partition-broadcast to [P, LC]
            abr_ps = stp.tile([1, 2 * NC, P], F32, tag="abr", name="abr_ps")
            for i in range(2):
                for j in range(NC):
                    nc.tensor.matmul(
                        abr_ps[:, i * NC + j, :], lhsT=ab0[c][:, i, j:j + 1],
                        rhs=ident_bf, start=True, stop=True)
            abr = esb.tile([1, 2, LC], BF16, tag=f"abrs{c}", name="abr")
            nc.scalar.copy(abr[:].rearrange("a b (j c2) -> a (b j) c2", c2=P),
                           abr_ps)
            psb = esb.tile([P, 2, LC], BF16, tag=f"psb{c}", name="psb")
            for i in range(2):
                nc.gpsimd.partition_broadcast(psb[:, i, :], abr[:, i, :])
            pab_sb.append(psb)
        # ---- LN0 apply + FFN ----
        for c in range(cfg.LCN):
            csl, psb = csls[c], pab_sb[c]
            for o in range(DO):
                xsl = X0[:, o, csl]
                nc.vector.tensor_tensor(xsl, O_bf[:, o, csl], psb[:, 0, :],
                                        OP.mult)
                nc.vector.tensor_tensor(xsl, xsl, psb[:, 1, :], OP.add)
                if cfg.ln0_gb:
                    nc.vector.scalar_tensor_tensor(
                        xsl, xsl, g0_pm[:, o:o + 1],
                        b0_pm[:, o:o + 1].to_broadcast([P, LC]),
                        OP.mult, OP.add)
        for c in range(cfg.LCN):
            csl = csls[c]
            for o in range(DO):
                fp = fps.tile([P, LC], F32, tag="f", name="fp")
                for ki in range(DO):
                    nc.tensor.matmul(fp, lhsT=woT[:, ki, o * P:(o + 1) * P],
                                     rhs=X0[:, ki, csl],
                                     start=(ki == 0), stop=(ki == DO - 1))
                h = esb.tile([P, LC], BF16, tag=f"h{o}{c}", name="h")
                nc.scalar.activation(h, fp, AF.Relu, bias=bo_pm[:, o:o + 1])
                nc.vector.tensor_tensor(X1[:, o, csl], X0[:, o, csl], h, OP.add)
        # ---- LN1 + transpose out ----
        st1 = [ln_stats(X1, csls[c], f"b{c}") for c in range(cfg.LCN)]
        ab1 = [ln_rows(st1[c], f"b{c}") for c in range(cfg.LCN)]
        for c in range(cfg.LCN):
            A81, B81 = ab1[c]
            for j in range(NC):
                t = c * NC + j
                tp = otp.tile([P, 2, P], BF16, tag="ot", name="tp")
                for o in range(DO):
                    nc.tensor.transpose(tp[:, o, :],
                                        X1[:, o, t * P:(t + 1) * P], ident_bf)
                ov = out_nat[:, t, :].rearrange("p (o c2) -> p o c2", c2=P)
                nc.vector.tensor_scalar(ov, tp, A81[:, j:j + 1], B81[:, j:j + 1],
                                        OP.mult, OP.add)
                if cfg.ln1_gb:
                    ovf = out_nat[:, t, :]
                    nc.vector.tensor_tensor(ovf, ovf, g1_bc, OP.mult)
                    nc.vector.tensor_tensor(ovf, ovf, b1_bc, OP.add)
            nc.sync.dma_start(
                io["out"][csls[c], :].rearrange("(t p) d -> p t d", p=P),
                out_nat[:, c * NC:(c + 1) * NC, :])


def build(cfg: Cfg) -> bass.Bass:
    nc = bacc.Bacc("TRN2")
    io = {}
    for name, shape, dt in (
        ("qT", [cfg.D, cfg.NQ], BF16), ("kT", [cfg.D, cfg.NK], BF16),
        ("WqT", [cfg.D, cfg.D], BF16), ("WkT", [cfg.D, cfg.D], BF16),
        ("WvT", [cfg.D, cfg.D], BF16), ("WoT", [cfg.D, cfg.D], BF16),
        ("bq", [cfg.D], F32), ("bk", [cfg.D], F32), ("bv", [cfg.D], F32),
        ("bo", [cfg.D], F32), ("g0", [cfg.D], F32), ("b0", [cfg.D], F32),
        ("g1", [cfg.D], F32), ("b1", [cfg.D], F32),
    ):
        io[name] = nc.dram_tensor(name, shape, dt, kind="ExternalInput")
    io["out"] = nc.dram_tensor("out", [cfg.NQ, cfg.D], F32, kind="ExternalOutput")

    with tile.TileContext(nc) as tc:
        with ExitStack() as ctx:
            _emit(nc, tc, ctx, io, cfg)
    nc.compile()
    return nc


_CACHE = {}


def _get_nc(key, cfg):
    if key not in _CACHE:
        _CACHE[key] = build(cfg)
    return _CACHE[key]


def kernel(q, k, Wq, bq, Wk, bk, Wv, bv, Wo, bo, g0, b0, g1, b1, _trace=False):
    from concourse.bass_utils import run_bass_kernel_spmd

    B, Nq, D = q.shape
    Nk = k.shape[1]
    n_cores = 8
    halves = n_cores // B
    nq_c = Nq // halves
    ln0_gb = not (np.allclose(g0, 1.0) and np.allclose(b0, 0.0))
    ln1_gb = not (np.allclose(g1, 1.0) and np.allclose(b1, 0.0))
    cfg = Cfg(NQ=nq_c, NK=Nk, D=D, ln0_gb=ln0_gb, ln1_gb=ln1_gb)
    nc = _get_nc((nq_c, Nk, D, ln0_gb, ln1_gb), cfg)

    bf = ml_dtypes.bfloat16

    def t_bf(a):  # [N, D] fp32 -> [D, N] bf16 contiguous
        return np.ascontiguousarray(np.asarray(a, np.float32).T.astype(bf))

    shared = dict(WqT=t_bf(Wq), WkT=t_bf(Wk), WvT=t_bf(Wv), WoT=t_bf(Wo))
    for n, v in (("bq", bq), ("bk", bk), ("bv", bv), ("bo", bo),
                 ("g0", g0), ("b0", b0), ("g1", g1), ("b1", b1)):
        shared[n] = np.ascontiguousarray(v, dtype=np.float32)
    kT = [t_bf(k[b]) for b in range(B)]
    in_maps = []
    for c in range(n_cores):
        b, hf = c // halves, c % halves
        m = dict(shared)
        m["qT"] = t_bf(q[b, hf * nq_c:(hf + 1) * nq_c])
        m["kT"] = kT[b]
        in_maps.append(m)

    res = run_bass_kernel_spmd(nc, in_maps, core_ids=list(range(n_cores)),
                               trace=_trace)
    out = np.empty((B, Nq, D), np.float32)
    for c in range(n_cores):
        b, hf = c // halves, c % halves
        out[b, hf * nq_c:(hf + 1) * nq_c] = res.results[c]["out"]
    if _trace:
        return out, res
    return out


# revision 21
# speedup vs baseline: 1.1659x; 1.1659x over previous
"""MAB (Set-Transformer multihead attention block) Trainium2 Bass kernel, v4.

Reference math (fp32):
  Q = q @ Wq.T + bq ; K = k @ Wk.T + bk ; V = k @ Wv.T + bv    [B,N,256]
  per head h (8 heads x 32): s = Qh @ Kh.T / 16 ; a = softmax(s)
  Oh = Qh + a @ Vh ; o = concat(Oh) ; o = LN0(o) ; o = o + relu(o @ Wo.T + bo)
  out = LN1(o)

Sharding: 8 cores = (batch b in 0..3, query-half in 0..1); no collectives.

v4 vs v3:
  - q/k/W are pre-transposed and pre-cast to bf16 on the host (input
    layout prep in the kernel() wrapper): no on-device input transposes
    or casts, and half the DMA bytes. Device inputs are qT/kT/W*T bf16.
  - epilogue rewritten: every matmul operand bf16 (no more fp32
    LOW_HIGH two-pass matmuls anywhere), LN0 broadcast tensors drained
    to SBUF bf16 so the apply runs at DVE 2x, and the two LN/FFN chunks
    are emitted stage-interleaved so their serial chains overlap.
  - attention unchanged from v3: 4-way row-tiled concurrent score
    matmuls, PV/denominator col-tiled matmuls interleaved at lag 1,
    exp split ACT (table exp) / DVE (bf16 bit-trick), denominators via
    ones matmuls, 1/d via reciprocal_approx_fast, rstd via rsqrt
    bit-trick + Newton (single ACT table load in the whole kernel).
"""

import os
import sys
from contextlib import ExitStack

import numpy as np

for _p in ("/opt/trn_rl_repo", "/root/.axon_site/_ro/trn_rl_repo"):
    if os.path.isdir(_p) and _p not in sys.path:
        sys.path.insert(0, _p)

import ml_dtypes  # noqa: E402
import concourse.bass as bass  # noqa: E402
import concourse.tile as tile  # noqa: E402
from concourse import bacc, mybir  # noqa: E402
from concourse.masks import make_identity  # noqa: E402

F32 = mybir.dt.float32
BF16 = mybir.dt.bfloat16
I16 = mybir.dt.int16
I32 = mybir.dt.int32
P = 128
EPS = 1e-5

AF = mybir.ActivationFunctionType
OP = mybir.AluOpType

LOG2E = 1.4426950408889634
# exp(x/16) ~= bf16(bits = round(x*EXP_A + EXP_B))
EXP_A = 128.0 * LOG2E / 16.0
EXP_B = 128.0 * (127.0 - 0.04380)
# rsqrt(v) ~= bf16(bits = round(bits32(v)*RS_A + RS_B)), then 1 Newton step
RS_A = -64.0 / (1 << 23)
RS_B = 24375.283445

ACT_EXP_SHARE = 0.58  # fraction of exp drains on ScalarE


class Cfg:
    def __init__(self, NQ=1024, NK=2048, D=256, H=8, ln0_gb=False, ln1_gb=False):
        self.NQ, self.NK, self.D, self.H = NQ, NK, D, H
        self.HD = D // H            # 32
        self.DO = D // P            # 2
        self.QT = NQ // P           # 8
        self.KT = NK // P           # 16
        self.SC = 512               # score/attn q-chunk
        self.QCN = NQ // self.SC    # 2
        self.LC = 512               # ln/ffn q-chunk
        self.LCN = NQ // self.LC
        self.ln0_gb = ln0_gb        # apply non-trivial g0/b0
        self.ln1_gb = ln1_gb
        assert self.HD == 32 and self.DO == 2


class EngineSplit:
    """Bresenham-style assigner: returns True for ACT with given share."""

    def __init__(self, share):
        self.share = share
        self.acc = 0.0

    def take(self):
        self.acc += self.share
        if self.acc >= 1.0:
            self.acc -= 1.0
            return True
        return False


def _emit(nc: bass.Bass, tc: tile.TileContext, ctx: ExitStack, io: dict, cfg: Cfg):
    NQ, NK, D, H = cfg.NQ, cfg.NK, cfg.D, cfg.H
    DO, QT, KT, SC, LC = cfg.DO, cfg.QT, cfg.KT, cfg.SC, cfg.LC

    const = ctx.enter_context(tc.tile_pool(name="const", bufs=1))
    persist = ctx.enter_context(tc.tile_pool(name="persist", bufs=1))

    # ---- constants ----
    ident = const.tile([P, P], F32)
    make_identity(nc, ident)
    ident_bf = const.tile([P, P], BF16)
    nc.vector.tensor_copy(ident_bf, ident)
    ones32 = const.tile([P, 32], BF16)
    nc.vector.memset(ones32, 1.0)
    ones_r = const.tile([1, 512], BF16)  # ones row: bias-matmul rhs / lhsT
    nc.vector.memset(ones_r, 1.0)
    ones_k = const.tile([P, 1], BF16)    # LN stats lhsT (partition sum)
    nc.vector.memset(ones_k, 1.0)

    def vec_row(name, dtype=BF16):
        tf = const.tile([1, D], F32, name=f"{name}_rowf")
        nc.sync.dma_start(tf, io[name][:].rearrange("(o d) -> o d", o=1))
        if dtype == F32:
            return tf
        t = const.tile([1, D], BF16, name=f"{name}_row")
        nc.vector.tensor_copy(t, tf)
        return t

    def vec_pm(name):
        t = const.tile([P, DO], F32, name=f"{name}_pm")
        nc.sync.dma_start(t, io[name][:].rearrange("(o p) -> p o", p=P))
        return t

    bq_pm, bk_pm, bo_pm = vec_pm("bq"), vec_pm("bk"), vec_pm("bo")
    bv_bc = const.tile([P, D], F32, name="bv_bc")
    nc.gpsimd.partition_broadcast(bv_bc, vec_row("bv", F32))
    g0_pm = vec_pm("g0") if cfg.ln0_gb else None
    b0_pm = vec_pm("b0") if cfg.ln0_gb else None
    if cfg.ln1_gb:
        g1_bc = const.tile([P, D], F32)
        nc.gpsimd.partition_broadcast(g1_bc, vec_row("g1", F32))
        b1_bc = const.tile([P, D], F32)
        nc.gpsimd.partition_broadcast(b1_bc, vec_row("b1", F32))

    # ---- persistent tensors (all bf16) ----
    Q_bf = persist.tile([P, DO, NQ], BF16, name="Q_bf")
    K_bf = persist.tile([P, DO, NK], BF16, name="K_bf")
    V_nat = persist.tile([P, KT, H, 32], BF16, name="V_nat")  # [tok, dv]/tile
    woT = persist.tile([P, DO, D], BF16, name="woT")
    O_bf = persist.tile([P, DO, NQ], BF16, name="O_bf")
    X0 = persist.tile([P, DO, NQ], BF16, name="X0")
    X1 = persist.tile([P, DO, NQ], BF16, name="X1")
    out_nat = persist.tile([P, QT, D], F32, name="out_nat")

    drain_split = EngineSplit(0.5)

    def drain(dst, src):
        if drain_split.take():
            nc.scalar.copy(dst, src)
        else:
            nc.vector.tensor_copy(dst, src)

    # ============ prologue: load pre-transposed inputs ============
    ph0 = ctx.enter_context(tc.tile_pool(name="ph0", bufs=1))

    # kT first (longest dependency chain: kT -> K/V proj -> attention)
    k_T = ph0.tile([P, DO, NK], BF16, name="k_T")
    for c0 in range(0, NK, NK // 2):
        nc.sync.dma_start(
            k_T[:, :, c0:c0 + NK // 2],
            io["kT"][:, c0:c0 + NK // 2].rearrange("(o p) t -> p o t", p=P))
    wT = {}
    for n in ("WkT", "WvT", "WqT", "WoT"):
        t = ph0.tile([P, DO, D], BF16, name=n) if n != "WoT" else woT
        nc.sync.dma_start(t, io[n][:].rearrange("(o p) f -> p o f", p=P))
        wT[n] = t
    q_T = ph0.tile([P, DO, NQ], BF16, name="q_T")
    nc.sync.dma_start(q_T, io["qT"][:].rearrange("(o p) t -> p o t", p=P))

    pdrain = EngineSplit(0.5)

    def proj_group(w, src, b_pm, dst, o, c0, ps, n=1024):
        for ki in range(DO):
            nc.tensor.matmul(
                ps, lhsT=w[:, ki, o * P:(o + 1) * P],
                rhs=src[:, ki, c0:c0 + n],
                start=(ki == 0), stop=(ki == DO - 1))
        if pdrain.take():
            nc.scalar.activation(dst[:, o, c0:c0 + n], ps, AF.Identity,
                                 bias=b_pm[:, o:o + 1])
        else:
            nc.vector.tensor_scalar(dst[:, o, c0:c0 + n], ps, 1.0,
                                    b_pm[:, o:o + 1], OP.mult, OP.add)

    # o=0 projections of K and Q gate the first score matmuls: do them
    # in a short serial pre-phase; everything else rides inside block 0.
    with tc.tile_pool(name="pps", bufs=1, space="PSUM") as pps:
        for c in range(4):
            ps = pps.tile([P, 512], F32, tag=f"pj{c % 2}", name="pj")
            proj_group(wT["WkT"], k_T, bk_pm, K_bf, 0, c * 512, ps, n=512)
        for c in range(2):
            ps = pps.tile([P, 512], F32, tag=f"pj{c % 2}", name="pj")
            proj_group(wT["WqT"], q_T, bq_pm, Q_bf, 0, c * 512, ps, n=512)

    # =================== attention ===================
    exp_split = EngineSplit(ACT_EXP_SHARE)
    with tc.tile_pool(name="attn_sb", bufs=1) as asb, \
         tc.tile_pool(name="sps", bufs=1, space="PSUM") as sps, \
         tc.tile_pool(name="ops", bufs=1, space="PSUM") as ops, \
         tc.tile_pool(name="nrm", bufs=2) as nrm:
        attn01 = asb.tile([P, KT, 2, SC], BF16, name="attn01")
        attn23 = asb.tile([P, KT, 2, SC], BF16, name="attn23")

        # leftover projections, emitted one group per kt-step of block 0
        # (they use the sp2 score buffers, which block 0 leaves free)
        extras = []
        for t in range(KT):
            def vproj(t=t):
                ps = sps.tile([P, 2, SC], F32, tag="sp2", name="pvv")
                psv = ps[:, 0, 0:D]
                for ki in range(DO):
                    nc.tensor.matmul(
                        psv, lhsT=k_T[:, ki, t * P:(t + 1) * P],
                        rhs=wT["WvT"][:, ki, :],
                        start=(ki == 0), stop=(ki == DO - 1))
                nc.vector.tensor_tensor(
                    V_nat[:, t, :, :].rearrange("p h w -> p (h w)"), psv,
                    bv_bc, OP.add)
            extras.append(vproj)
        for c in range(4):
            def kproj(c=c):
                ps = sps.tile([P, 2, SC], F32, tag="sp2", name="pvk")
                proj_group(wT["WkT"], k_T, bk_pm, K_bf, 1, c * 512,
                           ps[:, 1, :], n=512)
            extras.append(kproj)
        for c in range(2):
            def qproj(c=c):
                ps = sps.tile([P, 2, SC], F32, tag="sp2", name="pvq")
                proj_group(wT["WqT"], q_T, bq_pm, Q_bf, 1, c * 512,
                           ps[:, 1, :], n=512)
            extras.append(qproj)

        # flat (block, kt) pipeline: PV matmuls lag LAG steps behind the
        # score/exp stream, so the exp engines are never starved by a PV
        # tail and the PE fills exp-pacing gaps.
        LAG = 4
        pvq = []
        blocks = [(qc, o) for qc in range(cfg.QCN) for o in range(DO)]
        for bi, (qc, o) in enumerate(blocks):
            qsl = slice(qc * SC, (qc + 1) * SC)
            blk0 = bi == 0
            state = {}

            def pv_step(t, o=o, state=state):
                if "po" not in state:
                    state["po"] = ops.tile([P, SC], F32, tag="po", name="po")
                    state["pd"] = ops.tile([P, SC], F32, tag="pd", name="pd")
                po, pd = state["po"], state["pd"]
                for m in range(4):
                    at = attn01 if m < 2 else attn23
                    rhs = at[:, t, m % 2, :]
                    nc.tensor.matmul(
                        po[32 * m:32 * m + 32, :],
                        lhsT=V_nat[:, t, o * 4 + m, :], rhs=rhs,
                        start=(t == 0), stop=(t == KT - 1),
                        tile_position=(0, 32 * m), skip_group_check=True)
                for m in range(4):
                    at = attn01 if m < 2 else attn23
                    rhs = at[:, t, m % 2, :]
                    nc.tensor.matmul(
                        pd[32 * m:32 * m + 32, :], lhsT=ones32, rhs=rhs,
                        start=(t == 0), stop=(t == KT - 1),
                        tile_position=(0, 32 * m), skip_group_check=True)

            def norm(qsl=qsl, o=o, state=state):
                rec = nrm.tile([P, SC], F32, tag="rec", name="rec")
                nc.vector.reciprocal_approx_fast(rec, state["pd"])
                osl = O_bf[:, o, qsl]
                nc.vector.tensor_tensor(osl, state["po"], rec, OP.mult)
                nc.vector.tensor_tensor(osl, osl, Q_bf[:, o, qsl], OP.add)

            for kt in range(KT):
                if blk0:
                    extras.pop(0)()          # V tile kt
                    if extras and kt >= KT - 7:
                        extras.pop(0)()      # K/Q o=1 projections
                ksl = slice(kt * P, (kt + 1) * P)
                pj = (2 * kt) % (2 if blk0 else 3)
                pa = sps.tile([P, 2, SC], F32, tag=f"sp{pj}", name="pa")
                pb = sps.tile([P, 2, SC], F32,
                              tag=f"sp{(pj + 1) % (2 if blk0 else 3)}",
                              name="pb")
                for m, (pt, sl) in enumerate(
                        ((pa, 0), (pa, 1), (pb, 0), (pb, 1))):
                    nc.tensor.matmul(
                        pt[:, sl, :],
                        lhsT=K_bf[32 * m:32 * m + 32, o, ksl],
                        rhs=Q_bf[32 * m:32 * m + 32, o, qsl],
                        start=True, stop=True,
                        tile_position=(32 * m, 0))
                for at, pt in ((attn01, pa), (attn23, pb)):
                    if exp_split.take():
                        nc.scalar.activation(at[:, kt, :, :], pt, AF.Exp,
                                             scale=1.0 / 16.0)
                    else:
                        nc.vector.tensor_scalar(
                            at[:, kt, :, :].bitcast(I16), pt,
                            EXP_A, EXP_B, OP.mult, OP.add)
                if kt == KT - 1:
                    pvq.append(lambda t=kt, f=pv_step, n=norm: (f(t), n()))
                else:
                    pvq.append(lambda t=kt, f=pv_step: f(t))
                while len(pvq) > LAG:
                    pvq.pop(0)()
        while pvq:
            pvq.pop(0)()

    # =================== epilogue: LN0, FFN, LN1, out ===================
    # stage-interleaved across the two LC chunks so serial chains overlap
    with tc.tile_pool(name="ep_sb", bufs=1) as esb, \
         tc.tile_pool(name="st_ps", bufs=1, space="PSUM") as stp, \
         tc.tile_pool(name="bc_ps", bufs=1, space="PSUM") as bcp, \
         tc.tile_pool(name="f_ps", bufs=1, space="PSUM") as fps, \
         tc.tile_pool(name="o_ps", bufs=1, space="PSUM") as otp:
        NC = LC // P  # 4 query-pieces per chunk
        csls = [slice(c * LC, (c + 1) * LC) for c in range(cfg.LCN)]

        def ln_stats(src, csl, tag):
            """Stage 1: x^2 + ones-matmul stats -> st_sb [1, 2, LC] bf16."""
            x2 = esb.tile([P, DO, LC], BF16, tag=f"x2{tag}", name="x2")
            for o in range(DO):
                nc.scalar.activation(x2[:, o, :], src[:, o, csl], AF.Square)
            st_sb = esb.tile([1, 2, LC], BF16, tag=f"stsb{tag}", name="st_sb")
            for i in range(2):
                st = stp.tile([1, LC], F32, tag="st", name="st")
                for o in range(DO):
                    rhs = src[:, o, csl] if i == 0 else x2[:, o, :]
                    nc.tensor.matmul(st, lhsT=ones_k, rhs=rhs,
                                     start=(o == 0), stop=(o == DO - 1))
                nc.scalar.copy(st_sb[:, i, :], st)
            return st_sb

        def ln_rows(st_sb, tag, bf_out=False):
            """Stage 2: transpose stats pieces, row math -> A8/B8 [128, NC]."""
            st_t = stp.tile([P, 2 * NC], F32, tag="stt", name="st_t")
            for i in range(2):
                for j in range(NC):
                    nc.tensor.matmul(
                        st_t[:, i * NC + j:i * NC + j + 1],
                        lhsT=st_sb[0:1, i, j * P:(j + 1) * P],
                        rhs=ones_r[0:1, 0:1], start=True, stop=True)
            stt_sb = esb.tile([P, 2 * NC], F32, tag=f"sttsb{tag}", name="stt_sb")
            nc.vector.tensor_copy(stt_sb, st_t)
            sx, sx2 = stt_sb[:, 0:NC], stt_sb[:, NC:2 * NC]
            r8 = esb.tile([P, 5, NC], F32, tag=f"r8{tag}", name="r8")
            mu, ve, var_e, y2t, A8 = (r8[:, i, :] for i in range(5))
            nc.vector.tensor_scalar(mu, sx, 1.0 / D, None, OP.mult)
            nc.vector.tensor_scalar(ve, sx2, 1.0 / D, EPS, OP.mult, OP.add)
            nc.vector.tensor_tensor(var_e, mu, mu, OP.mult)
            nc.vector.tensor_tensor(var_e, ve, var_e, OP.subtract)
            y0 = esb.tile([P, NC], I16, tag=f"y0{tag}", name="y0")
            nc.vector.tensor_scalar(y0, var_e[:].bitcast(I32),
                                    RS_A, RS_B, OP.mult, OP.add)
            y0b = y0[:].bitcast(BF16)  # ~= rstd seed (+-3.7%)
            nc.vector.tensor_tensor(y2t, y0b, y0b, OP.mult)
            nc.vector.tensor_tensor(y2t, y2t, var_e, OP.mult)
            nc.vector.tensor_scalar(y2t, y2t, -0.5, 1.5, OP.mult, OP.add)
            nc.vector.tensor_tensor(A8, y2t, y0b, OP.mult)  # rstd
            B8 = r8[:, 3, :]  # reuse y2t slot
            nc.vector.scalar_tensor_tensor(B8, mu, -1.0, A8, OP.mult, OP.mult)
            if not bf_out:
                return A8, B8
            ab = esb.tile([P, 2, NC], BF16, tag=f"ab{tag}", name="ab")
            nc.vector.tensor_copy(ab[:, 0, :], A8)
            nc.vector.tensor_copy(ab[:, 1, :], B8)
            return ab

        # ---- LN0 ----
        st0 = [ln_stats(O_bf, csls[c], f"a{c}") for c in range(cfg.LCN)]
        ab0 = [ln_rows(st0[c], f"a{c}", bf_out=True) for c in range(cfg.LCN)]
        pab_sb = []
        for c in range(cfg.LCN):
            # transpose A/B pieces to partition-0 rows, then gpsimd
            # partition-broadcast to [P, LC]
            abr_ps = stp.tile([1, 2 * NC, P], F32, tag="abr", name="abr_ps")
            for i in range(2):
                for j in range(NC):
                    nc.tensor.matmul(
                        abr_ps[:, i * NC + j, :], lhsT=ab0[c][:, i, j:j + 1],
                        rhs=ident_bf, start=True, stop=True)
            abr = esb.tile([1, 2, LC], BF16, tag=f"abrs{c}", name="abr")
            nc.scalar.copy(abr[:].rearrange("a b (j c2) -> a (b j) c2", c2=P),
                           abr_ps)
            psb = esb.tile([P, 2, LC], BF16, tag=f"psb{c}", name="psb")
            for i in range(2):
                nc.gpsimd.partition_broadcast(psb[:, i, :], abr[:, i, :])
            pab_sb.append(psb)
        # ---- LN0 apply + FFN ----
        for c in range(cfg.LCN):
            csl, psb = csls[c], pab_sb[c]
            for o in range(DO):
                xsl = X0[:, o, csl]
                nc.vector.tensor_tensor(xsl, O_bf[:, o, csl], psb[:, 0, :],
                                        OP.mult)
                nc.vector.tensor_tensor(xsl, xsl, psb[:, 1, :], OP.add)
                if cfg.ln0_gb:
                    nc.vector.scalar_tensor_tensor(
                        xsl, xsl, g0_pm[:, o:o + 1],
                        b0_pm[:, o:o + 1].to_broadcast([P, LC]),
                        OP.mult, OP.add)
        for c in range(cfg.LCN):
            csl = csls[c]
            for o in range(DO):
                fp = fps.tile([P, LC], F32, tag="f", name="fp")
                for ki in range(DO):
                    nc.tensor.matmul(fp, lhsT=woT[:, ki, o * P:(o + 1) * P],
                                     rhs=X0[:, ki, csl],
                                     start=(ki == 0), stop=(ki == DO - 1))
                h = esb.tile([P, LC], BF16, tag=f"h{o}{c}", name="h")
                nc.scalar.activation(h, fp, AF.Relu, bias=bo_pm[:, o:o + 1])
                nc.vector.tensor_tensor(X1[:, o, csl], X0[:, o, csl], h, OP.add)
        # ---- LN1 + transpose out ----
        st1 = [ln_stats(X1, csls[c], f"b{c}") for c in range(cfg.LCN)]
        ab1 = [ln_rows(st1[c], f"b{c}") for c in range(cfg.LCN)]
        for c in range(cfg.LCN):
            A81, B81 = ab1[c]
            for j in range(NC):
                t = c * NC + j
                tp = otp.tile([P, 2, P], BF16, tag=f"ot{j % 2}", name="tp")
                for o in range(DO):
                    nc.tensor.transpose(tp[:, o, :],
                                        X1[:, o, t * P:(t + 1) * P], ident_bf)
                ov = out_nat[:, t, :].rearrange("p (o c2) -> p o c2", c2=P)
                if j % 2 == 0:
                    nc.vector.tensor_scalar(ov, tp, A81[:, j:j + 1],
                                            B81[:, j:j + 1], OP.mult, OP.add)
                else:
                    nc.scalar.activation(ov, tp, AF.Identity,
                                         bias=B81[:, j:j + 1],
                                         scale=A81[:, j:j + 1])
                if cfg.ln1_gb:
                    ovf = out_nat[:, t, :]
                    nc.vector.tensor_tensor(ovf, ovf, g1_bc, OP.mult)
                    nc.vector.tensor_tensor(ovf, ovf, b1_bc, OP.add)
            nc.sync.dma_start(
                io["out"][csls[c], :].rearrange("(t p) d -> p t d", p=P),
                out_nat[:, c * NC:(c + 1) * NC, :])


def build(cfg: Cfg) -> bass.Bass:
    nc = bacc.Bacc("TRN2")
    io = {}
    for name, shape, dt in (
        ("qT", [cfg.D, cfg.NQ], BF16), ("kT", [cfg.D, cfg.NK], BF16),
        ("WqT", [cfg.D, cfg.D], BF16), ("WkT", [cfg.D, cfg.D], BF16),
        ("WvT", [cfg.D, cfg.D], BF16), ("WoT", [cfg.D, cfg.D], BF16),
        ("bq", [cfg.D], F32), ("bk", [cfg.D], F32), ("bv", [cfg.D], F32),
        ("bo", [cfg.D], F32), ("g0", [cfg.D], F32), ("b0", [cfg.D], F32),
        ("g1", [cfg.D], F32), ("b1", [cfg.D], F32),
    ):
        io[name] = nc.dram_tensor(name, shape, dt, kind="ExternalInput")
    io["out"] = nc.dram_tensor("out", [cfg.NQ, cfg.D], F32, kind="ExternalOutput")

    with tile.TileContext(nc) as tc:
        with ExitStack() as ctx:
            _emit(nc, tc, ctx, io, cfg)
    nc.compile()
    return nc


_CACHE = {}


def _get_nc(key, cfg):
    if key not in _CACHE:
        _CACHE[key] = build(cfg)
    return _CACHE[key]


def kernel(q, k, Wq, bq, Wk, bk, Wv, bv, Wo, bo, g0, b0, g1, b1, _trace=False):
    from concourse.bass_utils import run_bass_kernel_spmd

    B, Nq, D = q.shape
    Nk = k.shape[1]
    n_cores = 8
    halves = n_cores // B
    nq_c = Nq // halves
    ln0_gb = not (np.allclose(g0, 1.0) and np.allclose(b0, 0.0))
    ln1_gb = not (np.allclose(g1, 1.0) and np.allclose(b1, 0.0))
    cfg = Cfg(NQ=nq_c, NK=Nk, D=D, ln0_gb=ln0_gb, ln1_gb=ln1_gb)
    nc = _get_nc((nq_c, Nk, D, ln0_gb, ln1_gb), cfg)

    bf = ml_dtypes.bfloat16

    def t_bf(a):  # [N, D] fp32 -> [D, N] bf16 contiguous
        return np.ascontiguousarray(np.asarray(a, np.float32).T.astype(bf))

    shared = dict(WqT=t_bf(Wq), WkT=t_bf(Wk), WvT=t_bf(Wv), WoT=t_bf(Wo))
    for n, v in (("bq", bq), ("bk", bk), ("bv", bv), ("bo", bo),
                 ("g0", g0), ("b0", b0), ("g1", g1), ("b1", b1)):
        shared[n] = np.ascontiguousarray(v, dtype=np.float32)
    kT = [t_bf(k[b]) for b in range(B)]
    in_maps = []
    for c in range(n_cores):
        b, hf = c // halves, c % halves
        m = dict(shared)
        m["qT"] = t_bf(q[b, hf * nq_c:(hf + 1) * nq_c])
        m["kT"] = kT[b]
        in_maps.append(m)

    res = run_bass_kernel_spmd(nc, in_maps, core_ids=list(range(n_cores)),
                               trace=_trace)
    out = np.empty((B, Nq, D), np.float32)
    for c in range(n_cores):
        b, hf = c // halves, c % halves
        out[b, hf * nq_c:(hf + 1) * nq_c] = res.results[c]["out"]
    if _trace:
        return out, res
    return out
